# revision 13
# baseline (speedup 1.0000x reference)
"""Trainium2 Bass kernel for nn_ASIC_87007447483060.

Math (reference restructure, validated to 8e-8 rel-l2 in fp32):
  rail = rail_state.reshape(2,2,1025,1025); rail[1,1,:n,0] = x
  v0 = rail[0,0,1:,1:]; v1 = rail[0,1,1:,1:]; v2 = rail[1,0,:n,:n]; v3 = rail[1,1,:n,:n]
  For i in 0..3 with (a,b,c) = indices of the other three planes:
    t_k = sigmoid(toggle_gates[i,k])             (8 planes)
    Horner soft-mux over the 3 "other" inputs (sum_k w_k == 1 identity):
      g_p = t_{2p}   + v_c (t_{2p+1} - t_{2p})   p=0..3
      h_q = g_{2q}   + v_b (g_{2q+1} - g_{2q})   q=0,1
      S   = h_0      + v_a (h_1 - h_0)
    out_i = (1-v_i) + (2 v_i - 1) S   in (0,1) exactly, so the reference's
    clip(0,1) never binds; fold the final scalar s = toggle_gates.flat[0]:
      out_i*s = Q_i S + R_i  with Q_i = s(2 v_i - 1), R_i = s(1 - v_i)
  Output = full rail with the four n x n computed planes scattered in and
  pass-through border values scaled by s (mask is all-ones).

Sharding: rows of the n x n grid split across 8 cores (128 rows each).
Because kernel() receives full inputs, the +/-1 halo is materialized on the
host by overlapping row slices -- no collectives needed.

Engine plan per core: DMA 16.8 MiB of toggle_gates (the memory roofline),
sigmoid batches on ScalarE (fp32 in -> fp16 out), Horner tree on VectorE in
fp16 (2x DVE mode), final add writes fp32.
"""

import os
import sys
from contextlib import ExitStack

for _p in (
    "/opt/trn_rl_repo",
    "/opt/pypackages",
    "/root/.axon_site/_ro/trn_rl_repo",
    "/root/.axon_site/_ro/pypackages",
):
    if os.path.isdir(_p) and _p not in sys.path:
        sys.path.append(_p)

import numpy as np  # noqa: E402

import concourse.tile as tile  # noqa: E402
from concourse import bacc, mybir  # noqa: E402
from concourse.bass_utils import run_bass_kernel_spmd  # noqa: E402

N = 1024
NCORES = 8
RPC = N // NCORES  # 128 rows per core
NPP = N + 1  # 1025
NB_VALS = 4 * (2 * NPP - 1)  # 8196 pass-through border values
NB_COLS = (NB_VALS + 127) // 128  # 65
OTHERS = ((1, 2, 3), (0, 2, 3), (0, 1, 3), (0, 1, 2))

f32 = mybir.dt.float32
f16 = mybir.dt.float16
AF = mybir.ActivationFunctionType

_BIDX = None
_NC = None


def _border_indices():
    """Flat rail indices of positions NOT overwritten by the 4 scatter regions."""
    idx = []
    P2 = NPP * NPP
    for plane, kind in (((0, 0), "lo"), ((0, 1), "lo"), ((1, 0), "hi"), ((1, 1), "hi")):
        a, b = plane
        base = (a * 2 + b) * P2
        if kind == "lo":  # computed region [0:N,0:N]: keep row N + col N (rows 0..N-1)
            idx.extend(base + N * NPP + c for c in range(NPP))
            idx.extend(base + r * NPP + N for r in range(N))
        else:  # computed region [1:,1:]: keep row 0 + col 0 (rows 1..N)
            idx.extend(base + c for c in range(NPP))
            idx.extend(base + r * NPP for r in range(1, NPP))
    return np.asarray(idx, np.int64)


def build_program():
    nc = bacc.Bacc("TRN2", debug=False, target_bir_lowering=False, num_devices=NCORES)
    tg = nc.dram_tensor("tg", [4, 8, RPC, N], f32, kind="ExternalInput").ap()
    vv = nc.dram_tensor("v", [4, RPC, N], f32, kind="ExternalInput").ap()
    sv = nc.dram_tensor("sv", [128, 1], f32, kind="ExternalInput").ap()
    bord = nc.dram_tensor("bord", [128, NB_COLS], f32, kind="ExternalInput").ap()
    out = nc.dram_tensor("out", [4, RPC, N], f32, kind="ExternalOutput").ap()
    bout = nc.dram_tensor("bout", [128, NB_COLS], f32, kind="ExternalOutput").ap()

    with tile.TileContext(nc) as tc, ExitStack() as ctx:
        const = ctx.enter_context(tc.tile_pool(name="const", bufs=1))
        tgp = ctx.enter_context(tc.tile_pool(name="tgp", bufs=5))
        tp = ctx.enter_context(tc.tile_pool(name="tp", bufs=2))
        hp = ctx.enter_context(tc.tile_pool(name="hp", bufs=1))
        op = ctx.enter_context(tc.tile_pool(name="op", bufs=2))

        # Queue split: v + tg loads on Sync HWDGE (v first -- its consumers
        # gate pipeline start); s/border loads and all stores (with
        # fp16->fp32 cast) on GPSIMD SWDGE so stores never block loads.
        # ACT runs a pure sigmoid stream; all prep runs on DVE in its idle
        # pre-window (before the first sigmoid lands).
        s_sb = const.tile([128, 1], f32, tag="s")
        nc.gpsimd.dma_start(s_sb[:], sv)
        two_s = const.tile([128, 1], f32, tag="2s")
        neg_s = const.tile([128, 1], f32, tag="-s")
        nc.vector.tensor_add(two_s[:], s_sb[:], s_sb[:])
        nc.vector.tensor_scalar_mul(neg_s[:], s_sb[:], -1.0)

        v32 = const.tile([128, 4 * N], f32, tag="v32")
        u16 = const.tile([128, 4 * N], f16, tag="u16")
        with tc.high_priority():
            # per-plane loads, v3 first: u16[3] is the first Horner multiplier
            for j in (3, 2, 1, 0):
                nc.sync.dma_start(v32[:, j * N : (j + 1) * N], vv[j])
        for j in (3, 2, 1, 0):
            nc.vector.tensor_copy(
                u16[:, j * N : (j + 1) * N], v32[:, j * N : (j + 1) * N]
            )
        Q16 = const.tile([128, 4 * N], f16, tag="Q16")
        nc.vector.tensor_scalar(
            Q16[:], v32[:], two_s[:], neg_s[:], mybir.AluOpType.mult, mybir.AluOpType.add
        )
        R16 = const.tile([128, 4 * N], f16, tag="R16")
        nc.vector.tensor_scalar(
            R16[:], v32[:], neg_s[:], s_sb[:], mybir.AluOpType.mult, mybir.AluOpType.add
        )

        bt = const.tile([128, NB_COLS], f32, tag="bt")
        nc.gpsimd.dma_start(bt[:], bord)
        bo = const.tile([128, NB_COLS], f32, tag="bo")
        nc.vector.tensor_scalar(
            bo[:], bt[:], s_sb[:], None, mybir.AluOpType.mult, mybir.AluOpType.bypass
        )
        nc.gpsimd.dma_start(bout, bo[:])

        # The 8-leaf Horner tree is separable per k-pair: levels 1+2 run
        # right behind each sigmoid chunk; only level 3 joins. i=0 streams
        # in 1MB quarter-chunks to start the pipeline early; i>0 in 2MB
        # halves for lower per-op overhead.
        for i in range(4):
            ja, jb, jc = OTHERS[i]
            uc = u16[:, jc * N : (jc + 1) * N]
            ub = u16[:, jb * N : (jb + 1) * N]
            g4 = hp.tile([128, 4 * N], f16, tag="g4")
            h2 = hp.tile([128, 2 * N], f16, tag="h2")
            def level2(hh):
                d2 = hp.tile([128, N], f16, tag="d2")
                nc.vector.tensor_sub(
                    d2[:],
                    g4[:, (2 * hh + 1) * N : (2 * hh + 2) * N],
                    g4[:, 2 * hh * N : (2 * hh + 1) * N],
                )
                m2 = hp.tile([128, N], f16, tag="m2")
                nc.vector.tensor_mul(m2[:], d2[:], ub)
                nc.vector.tensor_add(
                    h2[:, hh * N : (hh + 1) * N],
                    g4[:, 2 * hh * N : (2 * hh + 1) * N],
                    m2[:],
                )

            if i == 0:
                for p in range(4):  # one k-pair (1MB) at a time
                    tgq = tgp.tile([128, 2 * N], f32, tag="tg")
                    nc.sync.dma_start(
                        tgq[:].rearrange("p (k c) -> p k c", k=2),
                        tg[i, 2 * p : 2 * p + 2].rearrange("k p c -> p k c"),
                    )
                    t16q = tp.tile([128, 2 * N], f16, tag="t16")
                    nc.scalar.activation(t16q[:], tgq[:], AF.Sigmoid)
                    d1 = hp.tile([128, N], f16, tag="d")
                    nc.vector.tensor_sub(d1[:], t16q[:, N : 2 * N], t16q[:, 0:N])
                    m1 = hp.tile([128, N], f16, tag="m")
                    nc.vector.tensor_mul(m1[:], d1[:], uc)
                    nc.vector.tensor_add(
                        g4[:, p * N : (p + 1) * N], t16q[:, 0:N], m1[:]
                    )
                    if p % 2 == 1:
                        level2(p // 2)
            else:
                for hh in range(2):
                    tg32 = tgp.tile([128, 4 * N], f32, tag="tg")
                    nc.sync.dma_start(
                        tg32[:].rearrange("p (k c) -> p k c", k=4),
                        tg[i, 4 * hh : 4 * hh + 4].rearrange("k p c -> p k c"),
                    )
                    t16 = tp.tile([128, 4 * N], f16, tag="t16")
                    nc.scalar.activation(t16[:], tg32[:], AF.Sigmoid)

                    tt = t16[:].rearrange("p (k par c) -> p k par c", k=2, par=2)
                    d = hp.tile([128, 2 * N], f16, tag="d")
                    nc.vector.tensor_sub(
                        d[:].rearrange("p (k c) -> p k c", k=2),
                        tt[:, :, 1, :],
                        tt[:, :, 0, :],
                    )
                    m = hp.tile([128, 2 * N], f16, tag="m")
                    nc.vector.tensor_mul(m[:, 0:N], d[:, 0:N], uc)
                    nc.vector.tensor_mul(m[:, N : 2 * N], d[:, N : 2 * N], uc)
                    nc.vector.tensor_add(
                        g4[:, 2 * hh * N : (2 * hh + 2) * N].rearrange(
                            "p (k c) -> p k c", k=2
                        ),
                        tt[:, :, 0, :],
                        m[:].rearrange("p (k c) -> p k c", k=2),
                    )
                    level2(hh)

            d3 = hp.tile([128, N], f16, tag="d3")
            nc.vector.tensor_sub(d3[:], h2[:, N : 2 * N], h2[:, 0:N])
            m3 = hp.tile([128, N], f16, tag="m3")
            nc.vector.tensor_mul(m3[:], d3[:], u16[:, ja * N : (ja + 1) * N])
            S = hp.tile([128, N], f16, tag="S")
            nc.vector.tensor_add(S[:], h2[:, 0:N], m3[:])

            # final mix: out = Q_i*S + R_i
            mS = hp.tile([128, N], f16, tag="mS")
            nc.vector.tensor_mul(mS[:], S[:], Q16[:, i * N : (i + 1) * N])
            o16 = op.tile([128, N], f16, tag="o")
            nc.vector.tensor_add(o16[:], mS[:], R16[:, i * N : (i + 1) * N])
            # SWDGE store with fp16 -> fp32 cast, off the Sync load queue
            nc.gpsimd.dma_start(out[i], o16[:])

    nc.compile()
    return nc


def _get_program():
    global _NC
    if _NC is None:
        _NC = build_program()
    return _NC


def make_in_maps(x, toggle_gates, rail_state):
    """Host-side sharding: slice full inputs into the 8 per-core input maps."""
    global _BIDX
    if _BIDX is None:
        _BIDX = _border_indices()
    x = np.asarray(x, np.float32)
    tg = np.asarray(toggle_gates, np.float32)
    rail = np.asarray(rail_state, np.float32).reshape(2, 2, NPP, NPP).copy()
    rail[1, 1, :N, 0] = x  # the reference's view-write of x

    v = np.empty((4, N, N), np.float32)
    v[0] = rail[0, 0, 1:, 1:]
    v[1] = rail[0, 1, 1:, 1:]
    v[2] = rail[1, 0, :N, :N]
    v[3] = rail[1, 1, :N, :N]

    s = tg.reshape(-1)[0]
    sv = np.full((128, 1), s, np.float32)
    bord = np.zeros((128 * NB_COLS,), np.float32)
    bord[:NB_VALS] = rail.reshape(-1)[_BIDX]
    bord = bord.reshape(128, NB_COLS)

    in_maps = []
    for k in range(NCORES):
        r0 = k * RPC
        in_maps.append(
            {
                "tg": np.ascontiguousarray(tg[:, :, r0 : r0 + RPC, :]),
                "v": np.ascontiguousarray(v[:, r0 : r0 + RPC, :]),
                "sv": sv,
                "bord": bord,
            }
        )
    return in_maps


def assemble_output(results):
    """Host-side unshard: scatter per-core outputs back into the full rail."""
    outp = np.empty((2, 2, NPP, NPP), np.float32)
    for k in range(NCORES):
        r0 = k * RPC
        o = results[k]["out"]  # (4,128,1024), already scaled by s
        outp[0, 0, r0 : r0 + RPC, 0:N] = o[0]
        outp[0, 1, r0 : r0 + RPC, 0:N] = o[1]
        outp[1, 0, 1 + r0 : 1 + r0 + RPC, 1:NPP] = o[2]
        outp[1, 1, 1 + r0 : 1 + r0 + RPC, 1:NPP] = o[3]
    flat = outp.reshape(-1)
    flat[_BIDX] = results[0]["bout"].reshape(-1)[:NB_VALS]
    return flat


def run(x, toggle_gates, rail_state, mask, trace=False, tmpdir=None):
    in_maps = make_in_maps(x, toggle_gates, rail_state)
    nc = _get_program()
    res = run_bass_kernel_spmd(
        nc, in_maps, core_ids=list(range(NCORES)), trace=trace, tmpdir=tmpdir
    )
    flat = assemble_output(res.results)
    m = np.asarray(mask)
    if not (m == 1).all():  # spec fills mask with ones; identity multiply skipped
        flat = flat * m.astype(np.float32)
    return flat, res


def kernel(x, toggle_gates, rail_state, mask):
    flat, _ = run(x, toggle_gates, rail_state, mask)
    return flat


# revision 14
# speedup vs baseline: 1.1606x; 1.1606x over previous
"""Trainium2 Bass kernel for nn_ASIC_87007447483060.

Math (reference restructure, validated to 8e-8 rel-l2 in fp32):
  rail = rail_state.reshape(2,2,1025,1025); rail[1,1,:n,0] = x
  v0 = rail[0,0,1:,1:]; v1 = rail[0,1,1:,1:]; v2 = rail[1,0,:n,:n]; v3 = rail[1,1,:n,:n]
  For i in 0..3 with (a,b,c) = indices of the other three planes:
    t_k = sigmoid(toggle_gates[i,k])             (8 planes)
    Horner soft-mux over the 3 "other" inputs (sum_k w_k == 1 identity):
      g_p = t_{2p}   + v_c (t_{2p+1} - t_{2p})   p=0..3
      h_q = g_{2q}   + v_b (g_{2q+1} - g_{2q})   q=0,1
      S   = h_0      + v_a (h_1 - h_0)
    out_i = (1-v_i) + (2 v_i - 1) S   in (0,1) exactly, so the reference's
    clip(0,1) never binds; fold the final scalar s = toggle_gates.flat[0]:
      out_i*s = Q_i S + R_i  with Q_i = s(2 v_i - 1), R_i = s(1 - v_i)
  Output = full rail with the four n x n computed planes scattered in and
  pass-through border values scaled by s (mask is all-ones).

Sharding: rows of the n x n grid split across 8 cores (128 rows each).
Because kernel() receives full inputs, the +/-1 halo is materialized on the
host by overlapping row slices -- no collectives needed.

Engine plan per core: DMA 16.8 MiB of toggle_gates (the memory roofline),
sigmoid batches on ScalarE (fp32 in -> fp16 out), Horner tree on VectorE in
fp16 (2x DVE mode), final add writes fp32.
"""

import os
import sys
from contextlib import ExitStack

for _p in (
    "/opt/trn_rl_repo",
    "/opt/pypackages",
    "/root/.axon_site/_ro/trn_rl_repo",
    "/root/.axon_site/_ro/pypackages",
):
    if os.path.isdir(_p) and _p not in sys.path:
        sys.path.append(_p)

import numpy as np  # noqa: E402

import concourse.tile as tile  # noqa: E402
from concourse import bacc, mybir  # noqa: E402
from concourse.bass_utils import run_bass_kernel_spmd  # noqa: E402

N = 1024
NCORES = 8
RPC = N // NCORES  # 128 rows per core
NPP = N + 1  # 1025
NB_VALS = 4 * (2 * NPP - 1)  # 8196 pass-through border values
NB_COLS = (NB_VALS + 127) // 128  # 65
OTHERS = ((1, 2, 3), (0, 2, 3), (0, 1, 3), (0, 1, 2))

f32 = mybir.dt.float32
f16 = mybir.dt.float16
AF = mybir.ActivationFunctionType

_BIDX = None
_NC = None


def _border_indices():
    """Flat rail indices of positions NOT overwritten by the 4 scatter regions."""
    idx = []
    P2 = NPP * NPP
    for plane, kind in (((0, 0), "lo"), ((0, 1), "lo"), ((1, 0), "hi"), ((1, 1), "hi")):
        a, b = plane
        base = (a * 2 + b) * P2
        if kind == "lo":  # computed region [0:N,0:N]: keep row N + col N (rows 0..N-1)
            idx.extend(base + N * NPP + c for c in range(NPP))
            idx.extend(base + r * NPP + N for r in range(N))
        else:  # computed region [1:,1:]: keep row 0 + col 0 (rows 1..N)
            idx.extend(base + c for c in range(NPP))
            idx.extend(base + r * NPP for r in range(1, NPP))
    return np.asarray(idx, np.int64)


def build_program():
    nc = bacc.Bacc("TRN2", debug=False, target_bir_lowering=False, num_devices=NCORES)
    tg = nc.dram_tensor("tg", [4, 8, RPC, N], f32, kind="ExternalInput").ap()
    vv = nc.dram_tensor("v", [4, RPC, N], f32, kind="ExternalInput").ap()
    sv = nc.dram_tensor("sv", [128, 1], f32, kind="ExternalInput").ap()
    bord = nc.dram_tensor("bord", [128, NB_COLS], f32, kind="ExternalInput").ap()
    out = nc.dram_tensor("out", [4, RPC, N], f32, kind="ExternalOutput").ap()
    bout = nc.dram_tensor("bout", [128, NB_COLS], f32, kind="ExternalOutput").ap()

    AOP = mybir.AluOpType

    with tile.TileContext(nc) as tc, ExitStack() as ctx:
        const = ctx.enter_context(tc.tile_pool(name="const", bufs=1))
        tgp = ctx.enter_context(tc.tile_pool(name="tgp", bufs=4))
        tp = ctx.enter_context(tc.tile_pool(name="tp", bufs=2))
        hp = ctx.enter_context(tc.tile_pool(name="hp", bufs=1))
        op = ctx.enter_context(tc.tile_pool(name="op", bufs=2))

        # Loads ride the Sync HWDGE queue; in-flight DMAs share bandwidth
        # round-robin (credit depth ~4), so the first waves are small
        # (512KB per k-plane / v-plane) to get the pipeline started early.
        # Stores + tiny loads use the GPSIMD SWDGE queue (fp16->fp32 cast).
        s_sb = const.tile([128, 1], f32, tag="s")
        nc.gpsimd.dma_start(s_sb[:], sv)
        two_s = const.tile([128, 1], f32, tag="2s")
        neg_s = const.tile([128, 1], f32, tag="-s")
        nc.vector.tensor_add(two_s[:], s_sb[:], s_sb[:])
        nc.vector.tensor_scalar_mul(neg_s[:], s_sb[:], -1.0)
        bt = const.tile([128, NB_COLS], f32, tag="bt")
        nc.gpsimd.dma_start(bt[:], bord)

        v32 = const.tile([128, 4 * N], f32, tag="v32")
        u16 = const.tile([128, 4 * N], f16, tag="u16")
        Q16 = const.tile([128, 4 * N], f16, tag="Q16")
        R16 = const.tile([128, 4 * N], f16, tag="R16")

        def load_v(j):
            nc.sync.dma_start(v32[:, j * N : (j + 1) * N], vv[j])

        def cast_u(j, engine):
            src = v32[:, j * N : (j + 1) * N]
            dst = u16[:, j * N : (j + 1) * N]
            if engine == "act":
                nc.scalar.activation(dst, src, AF.Copy)
            else:
                nc.vector.tensor_copy(dst, src)

        # i=0 k-pair tiles, each filled by two 512KB plane-DMAs
        q_tiles = []
        for p in range(2):  # wave 1: pair0 planes + v3, v2
            tgq = tgp.tile([128, 2 * N], f32, tag="tg")
            nc.sync.dma_start(
                tgq[:, 0:N].rearrange("p (k c) -> p k c", k=1),
                tg[0, 2 * p : 2 * p + 1].rearrange("k p c -> p k c"),
            )
            nc.sync.dma_start(
                tgq[:, N : 2 * N].rearrange("p (k c) -> p k c", k=1),
                tg[0, 2 * p + 1 : 2 * p + 2].rearrange("k p c -> p k c"),
            )
            q_tiles.append(tgq)
            load_v(3 - p)
        for p in range(2, 4):  # wave 2: pair1 planes + v1, v0
            tgq = tgp.tile([128, 2 * N], f32, tag="tg")
            nc.sync.dma_start(
                tgq[:, 0:N].rearrange("p (k c) -> p k c", k=1),
                tg[0, 2 * p : 2 * p + 1].rearrange("k p c -> p k c"),
            )
            nc.sync.dma_start(
                tgq[:, N : 2 * N].rearrange("p (k c) -> p k c", k=1),
                tg[0, 2 * p + 1 : 2 * p + 2].rearrange("k p c -> p k c"),
            )
            q_tiles.append(tgq)
            load_v(3 - p)

        # DVE pre-window prep (DVE idle until the first sigmoid lands)
        cast_u(3, "dve")
        cast_u(2, "dve")
        bo = const.tile([128, NB_COLS], f32, tag="bo")
        nc.vector.tensor_scalar(bo[:], bt[:], s_sb[:], None, AOP.mult, AOP.bypass)
        nc.gpsimd.dma_start(bout, bo[:])

        def level2(i, g4, h2, hh):
            ub = u16[:, OTHERS[i][1] * N : (OTHERS[i][1] + 1) * N]
            d2 = hp.tile([128, N], f16, tag="d2")
            nc.vector.tensor_sub(
                d2[:],
                g4[:, (2 * hh + 1) * N : (2 * hh + 2) * N],
                g4[:, 2 * hh * N : (2 * hh + 1) * N],
            )
            m2 = hp.tile([128, N], f16, tag="m2")
            nc.vector.tensor_mul(m2[:], d2[:], ub)
            nc.vector.tensor_add(
                h2[:, hh * N : (hh + 1) * N],
                g4[:, 2 * hh * N : (2 * hh + 1) * N],
                m2[:],
            )

        def level3_mix(i, h2):
            ja = OTHERS[i][0]
            d3 = hp.tile([128, N], f16, tag="d3")
            nc.vector.tensor_sub(d3[:], h2[:, N : 2 * N], h2[:, 0:N])
            m3 = hp.tile([128, N], f16, tag="m3")
            nc.vector.tensor_mul(m3[:], d3[:], u16[:, ja * N : (ja + 1) * N])
            S = hp.tile([128, N], f16, tag="S")
            nc.vector.tensor_add(S[:], h2[:, 0:N], m3[:])
            mS = hp.tile([128, N], f16, tag="mS")
            nc.vector.tensor_mul(mS[:], S[:], Q16[:, i * N : (i + 1) * N])
            o16 = op.tile([128, N], f16, tag="o")
            nc.vector.tensor_add(o16[:], mS[:], R16[:, i * N : (i + 1) * N])
            nc.gpsimd.dma_start(out[i], o16[:])

        # ---- i = 0: quarter-granular Horner behind per-pair sigmoids ----
        g4 = hp.tile([128, 4 * N], f16, tag="g4")
        h2 = hp.tile([128, 2 * N], f16, tag="h2")
        uc0 = u16[:, 3 * N : 4 * N]
        for p in range(4):
            tgq = q_tiles[p]
            t16q = tp.tile([128, 2 * N], f16, tag="t16")
            nc.scalar.activation(t16q[:], tgq[:], AF.Sigmoid)
            if p == 0:
                cast_u(1, "act")  # slots into the ACT gap behind sigmoid p0
            if p == 1:
                cast_u(0, "act")
            d1 = hp.tile([128, N], f16, tag="d")
            nc.vector.tensor_sub(d1[:], t16q[:, N : 2 * N], t16q[:, 0:N])
            m1 = hp.tile([128, N], f16, tag="m")
            nc.vector.tensor_mul(m1[:], d1[:], uc0)
            nc.vector.tensor_add(g4[:, p * N : (p + 1) * N], t16q[:, 0:N], m1[:])
            if p == 1:
                level2(0, g4, h2, 0)
                nc.vector.tensor_scalar(
                    Q16[:], v32[:], two_s[:], neg_s[:], AOP.mult, AOP.add
                )
            if p == 3:
                level2(0, g4, h2, 1)
                nc.vector.tensor_scalar(
                    R16[:], v32[:], neg_s[:], s_sb[:], AOP.mult, AOP.add
                )
        level3_mix(0, h2)

        # ---- i = 1..3: 2MB halves ----
        for i in range(1, 4):
            jc = OTHERS[i][2]
            uc = u16[:, jc * N : (jc + 1) * N]
            g4 = hp.tile([128, 4 * N], f16, tag="g4")
            h2 = hp.tile([128, 2 * N], f16, tag="h2")
            for hh in range(2):
                tg32 = tgp.tile([128, 4 * N], f32, tag="tg")
                nc.sync.dma_start(
                    tg32[:].rearrange("p (k c) -> p k c", k=4),
                    tg[i, 4 * hh : 4 * hh + 4].rearrange("k p c -> p k c"),
                )
                t16 = tp.tile([128, 4 * N], f16, tag="t16")
                nc.scalar.activation(t16[:], tg32[:], AF.Sigmoid)

                tt = t16[:].rearrange("p (k par c) -> p k par c", k=2, par=2)
                d = hp.tile([128, 2 * N], f16, tag="d")
                nc.vector.tensor_sub(
                    d[:].rearrange("p (k c) -> p k c", k=2),
                    tt[:, :, 1, :],
                    tt[:, :, 0, :],
                )
                m = hp.tile([128, 2 * N], f16, tag="m")
                nc.vector.tensor_mul(
                    m[:].rearrange("p (k c) -> p k c", k=2),
                    d[:].rearrange("p (k c) -> p k c", k=2),
                    uc[:, None, :].broadcast_to((128, 2, N)),
                )
                nc.vector.tensor_add(
                    g4[:, 2 * hh * N : (2 * hh + 2) * N].rearrange(
                        "p (k c) -> p k c", k=2
                    ),
                    tt[:, :, 0, :],
                    m[:].rearrange("p (k c) -> p k c", k=2),
                )
                level2(i, g4, h2, hh)
            level3_mix(i, h2)

    nc.compile()
    return nc


def _get_program():
    global _NC
    if _NC is None:
        _NC = build_program()
    return _NC


def make_in_maps(x, toggle_gates, rail_state):
    """Host-side sharding: slice full inputs into the 8 per-core input maps."""
    global _BIDX
    if _BIDX is None:
        _BIDX = _border_indices()
    x = np.asarray(x, np.float32)
    tg = np.asarray(toggle_gates, np.float32)
    rail = np.asarray(rail_state, np.float32).reshape(2, 2, NPP, NPP).copy()
    rail[1, 1, :N, 0] = x  # the reference's view-write of x

    v = np.empty((4, N, N), np.float32)
    v[0] = rail[0, 0, 1:, 1:]
    v[1] = rail[0, 1, 1:, 1:]
    v[2] = rail[1, 0, :N, :N]
    v[3] = rail[1, 1, :N, :N]

    s = tg.reshape(-1)[0]
    sv = np.full((128, 1), s, np.float32)
    bord = np.zeros((128 * NB_COLS,), np.float32)
    bord[:NB_VALS] = rail.reshape(-1)[_BIDX]
    bord = bord.reshape(128, NB_COLS)

    in_maps = []
    for k in range(NCORES):
        r0 = k * RPC
        in_maps.append(
            {
                "tg": np.ascontiguousarray(tg[:, :, r0 : r0 + RPC, :]),
                "v": np.ascontiguousarray(v[:, r0 : r0 + RPC, :]),
                "sv": sv,
                "bord": bord,
            }
        )
    return in_maps


def assemble_output(results):
    """Host-side unshard: scatter per-core outputs back into the full rail."""
    outp = np.empty((2, 2, NPP, NPP), np.float32)
    for k in range(NCORES):
        r0 = k * RPC
        o = results[k]["out"]  # (4,128,1024), already scaled by s
        outp[0, 0, r0 : r0 + RPC, 0:N] = o[0]
        outp[0, 1, r0 : r0 + RPC, 0:N] = o[1]
        outp[1, 0, 1 + r0 : 1 + r0 + RPC, 1:NPP] = o[2]
        outp[1, 1, 1 + r0 : 1 + r0 + RPC, 1:NPP] = o[3]
    flat = outp.reshape(-1)
    flat[_BIDX] = results[0]["bout"].reshape(-1)[:NB_VALS]
    return flat


def run(x, toggle_gates, rail_state, mask, trace=False, tmpdir=None):
    in_maps = make_in_maps(x, toggle_gates, rail_state)
    nc = _get_program()
    res = run_bass_kernel_spmd(
        nc, in_maps, core_ids=list(range(NCORES)), trace=trace, tmpdir=tmpdir
    )
    flat = assemble_output(res.results)
    m = np.asarray(mask)
    if not (m == 1).all():  # spec fills mask with ones; identity multiply skipped
        flat = flat * m.astype(np.float32)
    return flat, res


def kernel(x, toggle_gates, rail_state, mask):
    flat, _ = run(x, toggle_gates, rail_state, mask)
    return flat
